# revision 33
# baseline (speedup 1.0000x reference)
"""Multi-head attention (B=4, S=2048, D=1024, H=16, dk=64) on 8 TRN2 NeuronCores.

Sharding: core c = (batch b = c//2, head-group g = c%2 of 8 heads).
Each core computes its head-group's attention output and the partial output
projection (Wo rows for its heads); the host sums the two partials per batch
and adds the (folded) output bias.

v2 design (all matmul operands bf16, PSUM accumulation f32):
  - Projections: QT/KT = W^T X^T (+bias) per (mt, block) tile;  V -> vaug
    tiles with an appended ones column per head (aug) so attn x V also
    yields the softmax denominators.
  - Scores: S^T tile pair per (hp, i): two K=64 matmuls (row groups 0/64)
    into one [128, 1024] PSUM tile; one EXP drains it to bf16 es.
  - U ("swapped" orientation): stationary = es slice [128 skv, 128 sq]
    (FWL bf16, LDW hidden), moving = vaug [128 skv, 65] ->
    psu[h][128 sq, t, 0:65] accumulated over i.  HW-measured ~31 ns/matmul
    vs 213 ns for the moving-es orientation -> U phase cost halves, and the
    denominator lands as a per-partition column (no partition broadcast).
  - Normalize on DVE (per-partition scalar mul), PE-transpose the [sq, hk]
    result back to O^T [hk, sq] via identity, out-proj per 128-col group.
  - Emission order software-pipelines everything: attention block j=0 starts
    after only K0.mt0+Q0.mt0; remaining K/V/Q blocks and the out-projection
    of block j-1 are woven into the ScalarE-bound attention phases.
    Q/K/V/vaug live in per-(mt, block) tiles so Tile's per-tile dependency
    tracking doesn't serialize attention behind later projection blocks.
ScalarE is the pacing engine: 256 EXPs x ~1.07us = ~274us floor.
"""

import numpy as np
import ml_dtypes

B, S, D = 4, 2048, 1024
H, DK = 16, 64
LH = 8                 # heads per core
HK = LH * DK           # 512 (local concat dim)
BLK = 512              # Sq block size
NB = S // BLK          # 4
ST = S // 128          # 16 Skv tiles
KT = D // 128          # 8 contraction tiles over D
MT = HK // 128         # 4 m-tiles over local heads

_CACHE = {}


def _build_program():
    from contextlib import ExitStack
    import concourse.tile as tile
    from concourse import bacc, mybir
    from concourse.tile_rust import add_dep_helper

    f32 = mybir.dt.float32
    bf16 = mybir.dt.bfloat16
    u16 = mybir.dt.uint16
    Exp = mybir.ActivationFunctionType.Exp

    nc = bacc.Bacc("TRN2", target_bir_lowering=False, debug=False, num_devices=8)

    xq_d = nc.dram_tensor("xq_t", [D, S], bf16, kind="ExternalInput")
    xk_d = nc.dram_tensor("xk_t", [D, S], bf16, kind="ExternalInput")
    xv_d = nc.dram_tensor("xv_t", [D, S], bf16, kind="ExternalInput")
    wq_d = nc.dram_tensor("wq", [D, HK], bf16, kind="ExternalInput")
    wk_d = nc.dram_tensor("wk", [D, HK], bf16, kind="ExternalInput")
    wv_d = nc.dram_tensor("wv", [D, HK], bf16, kind="ExternalInput")
    wo_d = nc.dram_tensor("wo", [HK, D], bf16, kind="ExternalInput")
    bq_d = nc.dram_tensor("bq2", [128, MT], f32, kind="ExternalInput")
    bk_d = nc.dram_tensor("bk2", [128, MT], f32, kind="ExternalInput")
    id_d = nc.dram_tensor("ident", [128, 128], bf16, kind="ExternalInput")
    y_d = nc.dram_tensor("y_t", [D, S], bf16, kind="ExternalOutput")

    with tile.TileContext(nc) as tc, ExitStack() as ctx:
        big = ctx.enter_context(tc.tile_pool(name="big", bufs=1))
        xsK = ctx.enter_context(tc.tile_pool(name="xsK", bufs=2))
        xsQ = ctx.enter_context(tc.tile_pool(name="xsQ", bufs=2))
        xsV = ctx.enter_context(tc.tile_pool(name="xsV", bufs=2))
        es_pool = ctx.enter_context(tc.tile_pool(name="es", bufs=14))
        ot_pool = ctx.enter_context(tc.tile_pool(name="ot", bufs=2))
        rpool = ctx.enter_context(tc.tile_pool(name="r", bufs=4))
        upool = ctx.enter_context(tc.tile_pool(name="u", bufs=3))
        ypool = ctx.enter_context(tc.tile_pool(name="y", bufs=4))
        # PSUM: psS 2x[128,1024] (4 banks) + psA 2x[128,4,128] (2) +
        #       psP 2x[128,512] (2) = 8 banks
        psS = ctx.enter_context(tc.tile_pool(name="psS", bufs=2, space="PSUM"))
        psA = ctx.enter_context(tc.tile_pool(name="psA", bufs=2, space="PSUM"))
        psP = ctx.enter_context(tc.tile_pool(name="psP", bufs=2, space="PSUM"))

        bq_sb = big.tile([128, MT], f32, tag="bq")
        bk_sb = big.tile([128, MT], f32, tag="bk")
        ident = big.tile([128, 128], bf16, tag="ident")
        nc.sync.dma_start(bq_sb[:], bq_d[:])
        nc.sync.dma_start(bk_sb[:], bk_d[:])
        nc.sync.dma_start(ident[:], id_d[:])

        # weight tiles; DMAs emitted in need-order below (wk/wq first, wo last)
        wk_sb = big.tile([128, KT, HK], bf16, tag="wk")
        wq_sb = big.tile([128, KT, HK], bf16, tag="wq")
        wv_sb = big.tile([128, KT, HK], bf16, tag="wv")
        wo_sb = big.tile([128, MT, D], bf16, tag="wo")
        nc.sync.dma_start(wk_sb[:], wk_d.ap().rearrange("(kt p) m -> p kt m", p=128))

        # per-(mt, block) tiles -> exact dependency granularity
        qtt = [[big.tile([128, BLK], bf16, tag=f"qt{mt}_{jb}",
                         name=f"qt{mt}_{jb}")
                for jb in range(NB)] for mt in range(MT)]
        ktt = [[big.tile([128, BLK], bf16, tag=f"kt{mt}_{jb}",
                         name=f"kt{mt}_{jb}")
                for jb in range(NB)] for mt in range(MT)]
        # V-aug per (Skv tile, head): full [128, DK+1] tiles so the U matmul
        # reads a whole tile (exact dependency on the drain, any granularity)
        vth = [big.tile([128, DK + 1], bf16, tag=f"vth{n}", name=f"vth{n}")
               for n in range(ST * LH)]
        for n in range(ST * LH):
            nc.vector.memset(vth[n][:, :].bitcast(u16), 0x3F80)

        def dma_x(pool, x_dram, j, name):
            xt = pool.tile([128, KT, BLK], bf16, tag="x", name=f"xt_{name}")
            nc.sync.dma_start(
                xt[:],
                x_dram.ap()[:, j * BLK : (j + 1) * BLK]
                .rearrange("(kt p) s -> p kt s", p=128),
            )
            return xt

        def proj_chain(xt, w_sb, bias_sb, dstt, j, mt, name):
            # single 8-deep chain into one psP slot, then bias-add drain
            pp = psP.tile([128, BLK], f32, tag="pp", name=f"pp_{name}")
            for kt in range(KT):
                nc.tensor.matmul(
                    pp[:], w_sb[:, kt, mt * 128 : (mt + 1) * 128], xt[:, kt, :],
                    start=(kt == 0), stop=(kt == KT - 1), skip_group_check=True,
                )
            nc.vector.tensor_scalar_add(
                dstt[mt][j][:], pp[:], bias_sb[:, mt : mt + 1]
            )

        def vproj_chain(xt, j, q, name):
            pp = psP.tile([128, BLK], f32, tag="pp", name=f"pp_v{name}")
            for kt in range(KT):
                nc.tensor.matmul(
                    pp[:], xt[:, kt, q * 128 : (q + 1) * 128], wv_sb[:, kt, :],
                    start=(kt == 0), stop=(kt == KT - 1), skip_group_check=True,
                )
            st = j * 4 + q
            for h in range(LH):
                nc.vector.tensor_copy(
                    vth[st * LH + h][:, 0:DK], pp[:, h * DK : (h + 1) * DK]
                )

        def emit_u(j, hp, i, es_t, psu):
            # stationary = es [128 skv, 128 sq] (FWL), moving = vaug [128, 65]
            # start=True on the bank's first matmul clears the whole bank's
            # has_written bits; the other regions' first writes then
            # overwrite-where-clear (HW-verified equivalent to pre-zeroing).
            for pi in range(2):
                h = 2 * hp + pi
                for t in range(4):
                    nc.tensor.matmul(
                        psu[pi][:, t, 0 : DK + 1],
                        es_t[:, (pi * 4 + t) * 128 : (pi * 4 + t + 1) * 128],
                        vth[i * LH + h][:, :],
                        start=(i == 0 and t == 0),
                        stop=(i == ST - 1),
                        skip_group_check=True,
                    )

        def att_block(j, otj):
            # generator: yields after each (hp, i) unit and after each norm
            for hp in range(MT):
                psu = [psA.tile([128, 4, 128], f32, tag="psA",
                                name=f"u{j}_{hp}_{p}") for p in range(2)]
                es_prev = None
                for i in range(ST):
                    ps2 = psS.tile([128, 2 * BLK], f32, tag="psS",
                                   name=f"s{j}_{hp}_{i}")
                    for pi in range(2):
                        bp = pi * 64
                        nc.tensor.matmul(
                            ps2[:, pi * BLK : (pi + 1) * BLK],
                            ktt[hp][i // 4][bp : bp + 64,
                                            (i % 4) * 128 : (i % 4 + 1) * 128],
                            qtt[hp][j][bp : bp + 64, :],
                            start=True, stop=True, skip_group_check=True,
                        )
                    es = es_pool.tile([128, 2 * BLK], bf16, tag="es")
                    nc.scalar.activation(es[:], ps2[:], Exp)
                    if es_prev is not None:
                        emit_u(j, hp, i - 1, es_prev, psu)
                    es_prev = es
                    yield
                emit_u(j, hp, ST - 1, es_prev, psu)
                # normalize: per-partition scalar 1/r, then transpose to O^T
                um = upool.tile([128, 4, 2, DK], bf16, tag="um",
                                name=f"um{j}_{hp}")
                for pi in range(2):
                    rf = rpool.tile([128, 4], f32, tag="rf",
                                    name=f"rf{j}_{hp}_{pi}")
                    nc.vector.reciprocal_approx_fast(
                        rf[:],
                        psu[pi][:, :, DK : DK + 1].rearrange("p a b -> p (a b)"),
                    )
                    for t in range(4):
                        nc.vector.tensor_scalar_mul(
                            um[:, t, pi, :], psu[pi][:, t, 0:DK], rf[:, t : t + 1]
                        )
                psT = psP.tile([128, 4, 128], bf16, tag="pp", name=f"pT{j}_{hp}")
                for t in range(4):
                    nc.tensor.transpose(
                        psT[:, t, :],
                        um[:, t, :, :].rearrange("p a b -> p (a b)"),
                        ident[:],
                    )
                nc.vector.tensor_copy(
                    otj[:, hp, :], psT[:].rearrange("p t c -> p (t c)")
                )
                yield

        def outproj_chunk(jprev, ot_prev, mos):
            for mo in mos:
                psy = psP.tile([128, BLK], f32, tag="pp", name=f"psy{jprev}_{mo}")
                for kt in range(MT):
                    nc.tensor.matmul(
                        psy[:], wo_sb[:, kt, mo * 128 : (mo + 1) * 128],
                        ot_prev[:, kt, :],
                        start=(kt == 0), stop=(kt == MT - 1),
                        skip_group_check=True,
                    )
                ysb = ypool.tile([128, BLK], bf16, tag="y", name=f"y{jprev}_{mo}")
                nc.vector.tensor_copy(ysb[:], psy[:])
                nc.sync.dma_start(
                    y_d[mo * 128 : (mo + 1) * 128,
                        jprev * BLK : (jprev + 1) * BLK], ysb[:]
                )

        def adv(g, n):
            for _ in range(n):
                next(g, None)

        # ---- fill: minimal path to the first exp (DMAs in need-order) ----
        xk0 = dma_x(xsK, xk_d, 0, "k0")
        nc.sync.dma_start(wq_sb[:], wq_d.ap().rearrange("(kt p) m -> p kt m", p=128))
        xq0 = dma_x(xsQ, xq_d, 0, "q0")
        proj_chain(xk0, wk_sb, bk_sb, ktt, 0, 0, "k0m0")
        proj_chain(xq0, wq_sb, bq_sb, qtt, 0, 0, "q0m0")
        nc.sync.dma_start(wv_sb[:], wv_d.ap().rearrange("(kt p) m -> p kt m", p=128))
        xv0 = dma_x(xsV, xv_d, 0, "v0")
        for mt in range(1, MT):
            proj_chain(xk0, wk_sb, bk_sb, ktt, 0, mt, f"k0m{mt}")
        for q in range(4):
            vproj_chain(xv0, 0, q, f"0q{q}")
        for mt in range(1, MT):
            proj_chain(xq0, wq_sb, bq_sb, qtt, 0, mt, f"q0m{mt}")

        ots = []
        ot0 = ot_pool.tile([128, MT, BLK], bf16, tag="ot", name="ot0")
        ots.append(ot0)
        g0 = att_block(0, ot0)

        adv(g0, 4)                       # hp0 i0..3
        # each K/V block must be EMITTED before the units that consume it
        # (scores i needs K(i//4); U i needs V(i//4)) or the reads bind to
        # the pre-drain data in program order.  K before V: the exp stream
        # only needs scores (K); U consumption lags via the es buffer, so
        # the scheduler runs K chains first and V fills PE slack after.
        xk1 = dma_x(xsK, xk_d, 1, "k1")
        for mt in range(MT):
            proj_chain(xk1, wk_sb, bk_sb, ktt, 1, mt, f"k1m{mt}")
        xv1 = dma_x(xsV, xv_d, 1, "v1")
        for q in range(4):
            vproj_chain(xv1, 1, q, f"1q{q}")
        adv(g0, 4)                       # hp0 i4..7
        xk2 = dma_x(xsK, xk_d, 2, "k2")
        for mt in range(MT):
            proj_chain(xk2, wk_sb, bk_sb, ktt, 2, mt, f"k2m{mt}")
        xv2 = dma_x(xsV, xv_d, 2, "v2")
        for q in range(4):
            vproj_chain(xv2, 2, q, f"2q{q}")
        adv(g0, 4)                       # hp0 i8..11
        xk3 = dma_x(xsK, xk_d, 3, "k3")
        for mt in range(MT):
            proj_chain(xk3, wk_sb, bk_sb, ktt, 3, mt, f"k3m{mt}")
        xv3 = dma_x(xsV, xv_d, 3, "v3")
        for q in range(4):
            vproj_chain(xv3, 3, q, f"3q{q}")
        adv(g0, 5)                       # hp0 i12..15 + norm
        adv(g0, 17)                      # hp1
        nc.sync.dma_start(wo_sb[:], wo_d.ap().rearrange("(kt p) m -> p kt m", p=128))
        xq1 = dma_x(xsQ, xq_d, 1, "q1")
        for mt in (0, 1):
            proj_chain(xq1, wq_sb, bq_sb, qtt, 1, mt, f"q1m{mt}")
        adv(g0, 17)                      # hp2
        for mt in (2, 3):
            proj_chain(xq1, wq_sb, bq_sb, qtt, 1, mt, f"q1m{mt}")
        adv(g0, 17)                      # hp3

        xqn = xq1
        for j in range(1, NB):
            otj = ot_pool.tile([128, MT, BLK], bf16, tag="ot", name=f"ot{j}")
            g = att_block(j, otj)
            for hp in range(MT):
                # weave in small pieces a few units into each hp group so
                # next-block scores never queue behind them on the PE
                adv(g, 3)
                outproj_chunk(j - 1, ots[j - 1], (2 * hp,))
                adv(g, 4)
                outproj_chunk(j - 1, ots[j - 1], (2 * hp + 1,))
                adv(g, 3)
                if j < NB - 1:
                    if hp == 0:
                        xqn = dma_x(xsQ, xq_d, j + 1, f"q{j+1}")
                    elif hp <= 2:
                        for mt in (0, 1) if hp == 1 else (2, 3):
                            proj_chain(xqn, wq_sb, bq_sb, qtt, j + 1, mt,
                                       f"q{j+1}m{mt}")
                adv(g, 7)
            ots.append(otj)
        outproj_chunk(NB - 1, ots[NB - 1], range(KT))

    nc.compile()
    return nc


def get_program():
    if "nc" not in _CACHE:
        _CACHE["nc"] = _build_program()
    return _CACHE["nc"]


def make_core_inputs(query, key, value, Wq, bq, Wk, bk, Wv, bv, Wo, bo):
    """Build the 8 per-core input dicts (and the folded output bias)."""
    f = np.float32
    bf = ml_dtypes.bfloat16
    ident = np.eye(128, dtype=bf)
    in_maps = []
    for c in range(8):
        b, g = c // 2, c % 2
        hs = slice(g * LH, (g + 1) * LH)
        m = {
            "xq_t": np.ascontiguousarray(query[b].T).astype(bf),
            "xk_t": np.ascontiguousarray(key[b].T).astype(bf),
            "xv_t": np.ascontiguousarray(value[b].T).astype(bf),
            "wq": np.ascontiguousarray(
                Wq[hs].transpose(1, 0, 2).reshape(D, HK) / 8.0
            ).astype(bf),
            "wk": np.ascontiguousarray(
                Wk[hs].transpose(1, 0, 2).reshape(D, HK)
            ).astype(bf),
            "wv": np.ascontiguousarray(
                Wv[hs].transpose(1, 0, 2).reshape(D, HK)
            ).astype(bf),
            "wo": np.ascontiguousarray(Wo[g * HK : (g + 1) * HK, :]).astype(bf),
            "bq2": np.ascontiguousarray(
                (bq[hs].reshape(HK) / 8.0).reshape(MT, 128).T, dtype=f
            ),
            "bk2": np.ascontiguousarray(
                bk[hs].reshape(HK).reshape(MT, 128).T, dtype=f
            ),
            "ident": ident,
        }
        in_maps.append(m)
    bo_eff = (bv.reshape(H * DK).astype(np.float64) @ Wo.astype(np.float64)
              + bo.astype(np.float64)).astype(f)
    return in_maps, bo_eff


def combine_outputs(results, bo_eff):
    """results: list of 8 dicts with 'y_t' [D, S] bf16. Returns [B, S, D] f32."""
    out = np.empty((B, S, D), dtype=np.float32)
    for b in range(B):
        acc = (results[2 * b]["y_t"].astype(np.float32)
               + results[2 * b + 1]["y_t"].astype(np.float32))
        out[b] = acc.T + bo_eff[None, :]
    return out


def kernel(**inputs):
    from concourse.bass_utils import run_bass_kernel_spmd

    inputs = {k: np.asarray(v) for k, v in inputs.items()}
    nc = get_program()
    in_maps, bo_eff = make_core_inputs(
        inputs["query"], inputs["key"], inputs["value"],
        inputs["Wq"], inputs["bq"], inputs["Wk"], inputs["bk"],
        inputs["Wv"], inputs["bv"], inputs["Wo"], inputs["bo"],
    )
    res = run_bass_kernel_spmd(nc, in_maps, list(range(8)))
    return combine_outputs(res.results, bo_eff)


# revision 36
# speedup vs baseline: 1.0019x; 1.0019x over previous
"""Multi-head attention (B=4, S=2048, D=1024, H=16, dk=64) on 8 TRN2 NeuronCores.

Sharding: core c = (batch b = c//2, head-group g = c%2 of 8 heads).
Each core computes its head-group's attention output and the partial output
projection (Wo rows for its heads); the host sums the two partials per batch
and adds the (folded) output bias.

v2 design (all matmul operands bf16, PSUM accumulation f32):
  - Projections: QT/KT = W^T X^T (+bias) per (mt, block) tile;  V -> vaug
    tiles with an appended ones column per head (aug) so attn x V also
    yields the softmax denominators.
  - Scores: S^T tile pair per (hp, i): two K=64 matmuls (row groups 0/64)
    into one [128, 1024] PSUM tile; one EXP drains it to bf16 es.
  - U ("swapped" orientation): stationary = es slice [128 skv, 128 sq]
    (FWL bf16, LDW hidden), moving = vaug [128 skv, 65] ->
    psu[h][128 sq, t, 0:65] accumulated over i.  HW-measured ~31 ns/matmul
    vs 213 ns for the moving-es orientation -> U phase cost halves, and the
    denominator lands as a per-partition column (no partition broadcast).
  - Normalize on DVE (per-partition scalar mul), PE-transpose the [sq, hk]
    result back to O^T [hk, sq] via identity, out-proj per 128-col group.
  - Emission order software-pipelines everything: attention block j=0 starts
    after only K0.mt0+Q0.mt0; remaining K/V/Q blocks and the out-projection
    of block j-1 are woven into the ScalarE-bound attention phases.
    Q/K/V/vaug live in per-(mt, block) tiles so Tile's per-tile dependency
    tracking doesn't serialize attention behind later projection blocks.
ScalarE is the pacing engine: 256 EXPs x ~1.07us = ~274us floor.
"""

import numpy as np
import ml_dtypes

B, S, D = 4, 2048, 1024
H, DK = 16, 64
LH = 8                 # heads per core
HK = LH * DK           # 512 (local concat dim)
BLK = 512              # Sq block size
NB = S // BLK          # 4
ST = S // 128          # 16 Skv tiles
KT = D // 128          # 8 contraction tiles over D
MT = HK // 128         # 4 m-tiles over local heads

_CACHE = {}


def _build_program():
    from contextlib import ExitStack
    import concourse.tile as tile
    from concourse import bacc, mybir
    from concourse.tile_rust import add_dep_helper

    f32 = mybir.dt.float32
    bf16 = mybir.dt.bfloat16
    u16 = mybir.dt.uint16
    Exp = mybir.ActivationFunctionType.Exp

    nc = bacc.Bacc("TRN2", target_bir_lowering=False, debug=False, num_devices=8)

    xq_d = nc.dram_tensor("xq_t", [D, S], bf16, kind="ExternalInput")
    xk_d = nc.dram_tensor("xk_t", [D, S], bf16, kind="ExternalInput")
    xv_d = nc.dram_tensor("xv_t", [D, S], bf16, kind="ExternalInput")
    wq_d = nc.dram_tensor("wq", [D, HK], bf16, kind="ExternalInput")
    wk_d = nc.dram_tensor("wk", [D, HK], bf16, kind="ExternalInput")
    wv_d = nc.dram_tensor("wv", [D, HK], bf16, kind="ExternalInput")
    wo_d = nc.dram_tensor("wo", [HK, D], bf16, kind="ExternalInput")
    bq_d = nc.dram_tensor("bq2", [128, MT], f32, kind="ExternalInput")
    bk_d = nc.dram_tensor("bk2", [128, MT], f32, kind="ExternalInput")
    id_d = nc.dram_tensor("ident", [128, 128], bf16, kind="ExternalInput")
    y_d = nc.dram_tensor("y_t", [D, S], bf16, kind="ExternalOutput")

    with tile.TileContext(nc) as tc, ExitStack() as ctx:
        big = ctx.enter_context(tc.tile_pool(name="big", bufs=1))
        xsK = ctx.enter_context(tc.tile_pool(name="xsK", bufs=2))
        xsQ = ctx.enter_context(tc.tile_pool(name="xsQ", bufs=2))
        xsV = ctx.enter_context(tc.tile_pool(name="xsV", bufs=2))
        es_pool = ctx.enter_context(tc.tile_pool(name="es", bufs=14))
        ot_pool = ctx.enter_context(tc.tile_pool(name="ot", bufs=2))
        rpool = ctx.enter_context(tc.tile_pool(name="r", bufs=4))
        upool = ctx.enter_context(tc.tile_pool(name="u", bufs=3))
        ypool = ctx.enter_context(tc.tile_pool(name="y", bufs=4))
        # PSUM: psS 2x[128,1024] (4 banks) + psA 2x[128,4,128] (2) +
        #       psP 2x[128,512] (2) = 8 banks
        psS = ctx.enter_context(tc.tile_pool(name="psS", bufs=2, space="PSUM"))
        psA = ctx.enter_context(tc.tile_pool(name="psA", bufs=2, space="PSUM"))
        psP = ctx.enter_context(tc.tile_pool(name="psP", bufs=2, space="PSUM"))

        bq_sb = big.tile([128, MT], f32, tag="bq")
        bk_sb = big.tile([128, MT], f32, tag="bk")
        ident = big.tile([128, 128], bf16, tag="ident")
        nc.sync.dma_start(bq_sb[:], bq_d[:])
        nc.sync.dma_start(bk_sb[:], bk_d[:])
        nc.sync.dma_start(ident[:], id_d[:])

        # weight tiles; DMAs emitted in need-order below (wk/wq first, wo last)
        wk_sb = big.tile([128, KT, HK], bf16, tag="wk")
        wq_sb = big.tile([128, KT, HK], bf16, tag="wq")
        wv_sb = big.tile([128, KT, HK], bf16, tag="wv")
        wo_sb = big.tile([128, MT, D], bf16, tag="wo")
        wk_src = wk_d.ap().rearrange("(kt p) m -> p kt m", p=128)
        nc.sync.dma_start(wk_sb[:, 0 : KT // 2, :], wk_src[:, 0 : KT // 2, :])
        nc.sync.dma_start(wk_sb[:, KT // 2 :, :], wk_src[:, KT // 2 :, :])

        # per-(mt, block) tiles -> exact dependency granularity
        qtt = [[big.tile([128, BLK], bf16, tag=f"qt{mt}_{jb}",
                         name=f"qt{mt}_{jb}")
                for jb in range(NB)] for mt in range(MT)]
        ktt = [[big.tile([128, BLK], bf16, tag=f"kt{mt}_{jb}",
                         name=f"kt{mt}_{jb}")
                for jb in range(NB)] for mt in range(MT)]
        # V-aug per (Skv tile, head): full [128, DK+1] tiles so the U matmul
        # reads a whole tile (exact dependency on the drain, any granularity)
        vth = [big.tile([128, DK + 1], bf16, tag=f"vth{n}", name=f"vth{n}")
               for n in range(ST * LH)]
        for n in range(ST * LH):
            nc.vector.memset(vth[n][:, :].bitcast(u16), 0x3F80)

        def dma_x(pool, x_dram, j, name):
            # two half-DMAs: dependency tracking is range-exact, so the
            # first 4 kt-steps of a consumer chain start after half arrives
            xt = pool.tile([128, KT, BLK], bf16, tag="x", name=f"xt_{name}")
            src = (x_dram.ap()[:, j * BLK : (j + 1) * BLK]
                   .rearrange("(kt p) s -> p kt s", p=128))
            nc.sync.dma_start(xt[:, 0 : KT // 2, :], src[:, 0 : KT // 2, :])
            nc.sync.dma_start(xt[:, KT // 2 :, :], src[:, KT // 2 :, :])
            return xt

        def proj_chain(xt, w_sb, bias_sb, dstt, j, mt, name):
            # single 8-deep chain into one psP slot, then bias-add drain
            pp = psP.tile([128, BLK], f32, tag="pp", name=f"pp_{name}")
            for kt in range(KT):
                nc.tensor.matmul(
                    pp[:], w_sb[:, kt, mt * 128 : (mt + 1) * 128], xt[:, kt, :],
                    start=(kt == 0), stop=(kt == KT - 1), skip_group_check=True,
                )
            nc.vector.tensor_scalar_add(
                dstt[mt][j][:], pp[:], bias_sb[:, mt : mt + 1]
            )

        def vproj_chain(xt, j, q, name):
            pp = psP.tile([128, BLK], f32, tag="pp", name=f"pp_v{name}")
            for kt in range(KT):
                nc.tensor.matmul(
                    pp[:], xt[:, kt, q * 128 : (q + 1) * 128], wv_sb[:, kt, :],
                    start=(kt == 0), stop=(kt == KT - 1), skip_group_check=True,
                )
            st = j * 4 + q
            for h in range(LH):
                nc.vector.tensor_copy(
                    vth[st * LH + h][:, 0:DK], pp[:, h * DK : (h + 1) * DK]
                )

        def emit_u(j, hp, i, es_t, psu):
            # stationary = es [128 skv, 128 sq] (FWL), moving = vaug [128, 65]
            # start=True on the bank's first matmul clears the whole bank's
            # has_written bits; the other regions' first writes then
            # overwrite-where-clear (HW-verified equivalent to pre-zeroing).
            for pi in range(2):
                h = 2 * hp + pi
                for t in range(4):
                    nc.tensor.matmul(
                        psu[pi][:, t, 0 : DK + 1],
                        es_t[:, (pi * 4 + t) * 128 : (pi * 4 + t + 1) * 128],
                        vth[i * LH + h][:, :],
                        start=(i == 0 and t == 0),
                        stop=(i == ST - 1),
                        skip_group_check=True,
                    )

        def att_block(j, otj):
            # generator: yields after each (hp, i) unit and after each norm
            for hp in range(MT):
                psu = [psA.tile([128, 4, 128], f32, tag="psA",
                                name=f"u{j}_{hp}_{p}") for p in range(2)]
                es_prev = None
                for i in range(ST):
                    ps2 = psS.tile([128, 2 * BLK], f32, tag="psS",
                                   name=f"s{j}_{hp}_{i}")
                    for pi in range(2):
                        bp = pi * 64
                        nc.tensor.matmul(
                            ps2[:, pi * BLK : (pi + 1) * BLK],
                            ktt[hp][i // 4][bp : bp + 64,
                                            (i % 4) * 128 : (i % 4 + 1) * 128],
                            qtt[hp][j][bp : bp + 64, :],
                            start=True, stop=True, skip_group_check=True,
                        )
                    es = es_pool.tile([128, 2 * BLK], bf16, tag="es")
                    nc.scalar.activation(es[:], ps2[:], Exp)
                    if es_prev is not None:
                        emit_u(j, hp, i - 1, es_prev, psu)
                    es_prev = es
                    yield
                emit_u(j, hp, ST - 1, es_prev, psu)
                # normalize: per-partition scalar 1/r, then transpose to O^T
                um = upool.tile([128, 4, 2, DK], bf16, tag="um",
                                name=f"um{j}_{hp}")
                for pi in range(2):
                    rf = rpool.tile([128, 4], f32, tag="rf",
                                    name=f"rf{j}_{hp}_{pi}")
                    nc.vector.reciprocal_approx_fast(
                        rf[:],
                        psu[pi][:, :, DK : DK + 1].rearrange("p a b -> p (a b)"),
                    )
                    for t in range(4):
                        nc.vector.tensor_scalar_mul(
                            um[:, t, pi, :], psu[pi][:, t, 0:DK], rf[:, t : t + 1]
                        )
                psT = psP.tile([128, 4, 128], bf16, tag="pp", name=f"pT{j}_{hp}")
                for t in range(4):
                    nc.tensor.transpose(
                        psT[:, t, :],
                        um[:, t, :, :].rearrange("p a b -> p (a b)"),
                        ident[:],
                    )
                nc.vector.tensor_copy(
                    otj[:, hp, :], psT[:].rearrange("p t c -> p (t c)")
                )
                yield

        def outproj_chunk(jprev, ot_prev, mos):
            for mo in mos:
                psy = psP.tile([128, BLK], f32, tag="pp", name=f"psy{jprev}_{mo}")
                for kt in range(MT):
                    nc.tensor.matmul(
                        psy[:], wo_sb[:, kt, mo * 128 : (mo + 1) * 128],
                        ot_prev[:, kt, :],
                        start=(kt == 0), stop=(kt == MT - 1),
                        skip_group_check=True,
                    )
                ysb = ypool.tile([128, BLK], bf16, tag="y", name=f"y{jprev}_{mo}")
                nc.vector.tensor_copy(ysb[:], psy[:])
                nc.sync.dma_start(
                    y_d[mo * 128 : (mo + 1) * 128,
                        jprev * BLK : (jprev + 1) * BLK], ysb[:]
                )

        def adv(g, n):
            for _ in range(n):
                next(g, None)

        # ---- fill: minimal path to the first exp (DMAs in need-order) ----
        xk0 = dma_x(xsK, xk_d, 0, "k0")
        wq_src = wq_d.ap().rearrange("(kt p) m -> p kt m", p=128)
        nc.sync.dma_start(wq_sb[:, 0 : KT // 2, :], wq_src[:, 0 : KT // 2, :])
        nc.sync.dma_start(wq_sb[:, KT // 2 :, :], wq_src[:, KT // 2 :, :])
        xq0 = dma_x(xsQ, xq_d, 0, "q0")
        proj_chain(xk0, wk_sb, bk_sb, ktt, 0, 0, "k0m0")
        proj_chain(xq0, wq_sb, bq_sb, qtt, 0, 0, "q0m0")
        nc.sync.dma_start(wv_sb[:], wv_d.ap().rearrange("(kt p) m -> p kt m", p=128))
        xv0 = dma_x(xsV, xv_d, 0, "v0")
        for mt in range(1, MT):
            proj_chain(xk0, wk_sb, bk_sb, ktt, 0, mt, f"k0m{mt}")
        for q in range(4):
            vproj_chain(xv0, 0, q, f"0q{q}")
        for mt in range(1, MT):
            proj_chain(xq0, wq_sb, bq_sb, qtt, 0, mt, f"q0m{mt}")

        ots = []
        ot0 = ot_pool.tile([128, MT, BLK], bf16, tag="ot", name="ot0")
        ots.append(ot0)
        g0 = att_block(0, ot0)

        adv(g0, 4)                       # hp0 i0..3
        # each K/V block must be EMITTED before the units that consume it
        # (scores i needs K(i//4); U i needs V(i//4)) or the reads bind to
        # the pre-drain data in program order.  K before V: the exp stream
        # only needs scores (K); U consumption lags via the es buffer, so
        # the scheduler runs K chains first and V fills PE slack after.
        xk1 = dma_x(xsK, xk_d, 1, "k1")
        for mt in range(MT):
            proj_chain(xk1, wk_sb, bk_sb, ktt, 1, mt, f"k1m{mt}")
        xv1 = dma_x(xsV, xv_d, 1, "v1")
        for q in range(4):
            vproj_chain(xv1, 1, q, f"1q{q}")
        adv(g0, 4)                       # hp0 i4..7
        xk2 = dma_x(xsK, xk_d, 2, "k2")
        for mt in range(MT):
            proj_chain(xk2, wk_sb, bk_sb, ktt, 2, mt, f"k2m{mt}")
        xv2 = dma_x(xsV, xv_d, 2, "v2")
        for q in range(4):
            vproj_chain(xv2, 2, q, f"2q{q}")
        adv(g0, 4)                       # hp0 i8..11
        xk3 = dma_x(xsK, xk_d, 3, "k3")
        for mt in range(MT):
            proj_chain(xk3, wk_sb, bk_sb, ktt, 3, mt, f"k3m{mt}")
        xv3 = dma_x(xsV, xv_d, 3, "v3")
        for q in range(4):
            vproj_chain(xv3, 3, q, f"3q{q}")
        adv(g0, 5)                       # hp0 i12..15 + norm
        adv(g0, 17)                      # hp1
        nc.sync.dma_start(wo_sb[:], wo_d.ap().rearrange("(kt p) m -> p kt m", p=128))
        xq1 = dma_x(xsQ, xq_d, 1, "q1")
        for mt in (0, 1):
            proj_chain(xq1, wq_sb, bq_sb, qtt, 1, mt, f"q1m{mt}")
        adv(g0, 17)                      # hp2
        for mt in (2, 3):
            proj_chain(xq1, wq_sb, bq_sb, qtt, 1, mt, f"q1m{mt}")
        adv(g0, 17)                      # hp3

        xqn = xq1
        for j in range(1, NB):
            otj = ot_pool.tile([128, MT, BLK], bf16, tag="ot", name=f"ot{j}")
            g = att_block(j, otj)
            for hp in range(MT):
                # weave in small pieces a few units into each hp group so
                # next-block scores never queue behind them on the PE
                adv(g, 3)
                outproj_chunk(j - 1, ots[j - 1], (2 * hp,))
                adv(g, 4)
                outproj_chunk(j - 1, ots[j - 1], (2 * hp + 1,))
                adv(g, 3)
                if j < NB - 1:
                    if hp == 0:
                        xqn = dma_x(xsQ, xq_d, j + 1, f"q{j+1}")
                    elif hp <= 2:
                        for mt in (0, 1) if hp == 1 else (2, 3):
                            proj_chain(xqn, wq_sb, bq_sb, qtt, j + 1, mt,
                                       f"q{j+1}m{mt}")
                adv(g, 7)
            ots.append(otj)
        outproj_chunk(NB - 1, ots[NB - 1], range(KT))

    nc.compile()
    return nc


def get_program():
    if "nc" not in _CACHE:
        _CACHE["nc"] = _build_program()
    return _CACHE["nc"]


def make_core_inputs(query, key, value, Wq, bq, Wk, bk, Wv, bv, Wo, bo):
    """Build the 8 per-core input dicts (and the folded output bias)."""
    f = np.float32
    bf = ml_dtypes.bfloat16
    ident = np.eye(128, dtype=bf)
    in_maps = []
    for c in range(8):
        b, g = c // 2, c % 2
        hs = slice(g * LH, (g + 1) * LH)
        m = {
            "xq_t": np.ascontiguousarray(query[b].T).astype(bf),
            "xk_t": np.ascontiguousarray(key[b].T).astype(bf),
            "xv_t": np.ascontiguousarray(value[b].T).astype(bf),
            "wq": np.ascontiguousarray(
                Wq[hs].transpose(1, 0, 2).reshape(D, HK) / 8.0
            ).astype(bf),
            "wk": np.ascontiguousarray(
                Wk[hs].transpose(1, 0, 2).reshape(D, HK)
            ).astype(bf),
            "wv": np.ascontiguousarray(
                Wv[hs].transpose(1, 0, 2).reshape(D, HK)
            ).astype(bf),
            "wo": np.ascontiguousarray(Wo[g * HK : (g + 1) * HK, :]).astype(bf),
            "bq2": np.ascontiguousarray(
                (bq[hs].reshape(HK) / 8.0).reshape(MT, 128).T, dtype=f
            ),
            "bk2": np.ascontiguousarray(
                bk[hs].reshape(HK).reshape(MT, 128).T, dtype=f
            ),
            "ident": ident,
        }
        in_maps.append(m)
    bo_eff = (bv.reshape(H * DK).astype(np.float64) @ Wo.astype(np.float64)
              + bo.astype(np.float64)).astype(f)
    return in_maps, bo_eff


def combine_outputs(results, bo_eff):
    """results: list of 8 dicts with 'y_t' [D, S] bf16. Returns [B, S, D] f32."""
    out = np.empty((B, S, D), dtype=np.float32)
    for b in range(B):
        acc = (results[2 * b]["y_t"].astype(np.float32)
               + results[2 * b + 1]["y_t"].astype(np.float32))
        out[b] = acc.T + bo_eff[None, :]
    return out


def kernel(**inputs):
    from concourse.bass_utils import run_bass_kernel_spmd

    inputs = {k: np.asarray(v) for k, v in inputs.items()}
    nc = get_program()
    in_maps, bo_eff = make_core_inputs(
        inputs["query"], inputs["key"], inputs["value"],
        inputs["Wq"], inputs["bq"], inputs["Wk"], inputs["bk"],
        inputs["Wv"], inputs["bv"], inputs["Wo"], inputs["bo"],
    )
    res = run_bass_kernel_spmd(nc, in_maps, list(range(8)))
    return combine_outputs(res.results, bo_eff)
